# revision 13
# baseline (speedup 1.0000x reference)
"""DA3 CrossFrame CF Angle Loss — Trainium2 Bass kernel (8-core SPMD).

Sharding: sim/topk phase sharded over the 8192 extra rows (E); angle
phase sharded over the 512 ref rows (R). One AllToAll exchanges per-ref
top-4 candidates. Per-core partial sums [3] are combined on the host.

v2: host-normalized ext (no on-device gram/recip/broadcast), host ss/rr
norms, packed DMA layouts (8-16KB rows), transposed dma_gather for the
h^T matmul operand (no TensorE transposes), scalar-engine table Rsqrt
(replaces DVE iterative reciprocal), sr matmuls overlap the AllToAll.
"""

import numpy as np
import ml_dtypes

import concourse.bass as bass
import concourse.bacc as bacc
import concourse.mybir as mybir
import concourse.bass_isa as bass_isa
from concourse.tile import TileContext
from concourse.bass_utils import run_bass_kernel_spmd
from concourse import library_config

F32 = mybir.dt.float32
BF16 = mybir.dt.bfloat16
I16 = mybir.dt.int16
I32 = mybir.dt.int32
U32 = mybir.dt.uint32
AF = mybir.ActivationFunctionType
OP = mybir.AluOpType
AX = mybir.AxisListType

NC_N = 8
B, P, D = 2, 2048, 1024
R = S = 512
K = 4
E = 4 * P          # 8192
ESH = E // NC_N    # 1024
RSH = R // NC_N    # 64
RK = RSH * K       # 256  (j = k*RSH + r, k outer)
DC = D // 128      # 8
NFP = 3            # frame pairs

RSQRT_NR = False   # one Newton-Raphson step after the table rsqrt


def raw_activation(nc, out, in_, func, bias=0.0, scale=1.0, accum_out=None):
    """nc.scalar.activation without the Rsqrt/Reciprocal accuracy guard."""
    se = nc.scalar
    if isinstance(bias, float):
        bias = nc.const_aps.scalar_like(bias, in_)
    inputs = [se.lower_ap(in_)]
    for arg in (bias, scale, 0.0):
        if isinstance(arg, bass.AP):
            inputs.append(se.lower_ap(arg))
        else:
            inputs.append(mybir.ImmediateValue(dtype=mybir.dt.float32, value=arg))
    outputs = [se.lower_ap(out)]
    if accum_out is not None:
        outputs.append(se.lower_ap(accum_out))
    return se.add_instruction(
        mybir.InstActivation(
            name=nc.get_next_instruction_name(),
            func=func,
            ins=inputs,
            outs=outputs,
        )
    )


def build(debug=False):
    nc = bacc.Bacc("TRN2", target_bir_lowering=False, debug=False,
                   num_devices=NC_N)

    T = {}
    T["extTn"] = nc.dram_tensor("extTn", [B, 128, DC * ESH], BF16, kind="ExternalInput")
    T["refTt"] = nc.dram_tensor("refTt", [B, 128, DC * R], BF16, kind="ExternalInput")
    T["refoT"] = nc.dram_tensor("refoT", [2, B, 128, DC * 128], BF16, kind="ExternalInput")
    T["refnat"] = nc.dram_tensor("refnat", [2, B, 128, D], BF16, kind="ExternalInput")
    T["shT"] = nc.dram_tensor("shT", [NFP, 2, B, 128, DC * S], BF16, kind="ExternalInput")
    T["extrows"] = nc.dram_tensor("extrows", [B, E, D], BF16, kind="ExternalInput")
    T["ss12"] = nc.dram_tensor("ss12", [2, 12 * 512], BF16, kind="ExternalInput")
    T["mhalf"] = nc.dram_tensor("mhalf", [2, 128], BF16, kind="ExternalInput")
    T["offtab"] = nc.dram_tensor("offtab", [128, 32], I32, kind="ExternalInput")
    T["rrep"] = nc.dram_tensor("rrep", [128, 4], F32, kind="ExternalInput")
    T["partials"] = nc.dram_tensor("partials", [1, 4], F32, kind="ExternalOutput")
    dbg = {}
    if debug:
        dbg["d_sim"] = nc.dram_tensor("d_sim", [B, 128, ESH], F32, kind="ExternalOutput")
        dbg["d_vi"] = nc.dram_tensor("d_vi", [128, 64], F32, kind="ExternalOutput")
        dbg["d_win"] = nc.dram_tensor("d_win", [128, 4], F32, kind="ExternalOutput")
        dbg["d_go"] = nc.dram_tensor("d_go", [B, 128, 2 * D], F32, kind="ExternalOutput")
        dbg["d_hT"] = nc.dram_tensor("d_hT", [B, 128, DC * RK], F32, kind="ExternalOutput")
        dbg["d_scal"] = nc.dram_tensor("d_scal", [128, 64], F32, kind="ExternalOutput")
        dbg["d_u1"] = nc.dram_tensor("d_u1", [2, 128, 1024], F32, kind="ExternalOutput")
        dbg["d_sr2"] = nc.dram_tensor("d_sr2", [2, 128, 1024], F32, kind="ExternalOutput")
        dbg["d_a"] = nc.dram_tensor("d_a", [3, 2, 128, 1024], F32, kind="ExternalOutput")
        dbg["d_acc"] = nc.dram_tensor("d_acc", [128, 20], F32, kind="ExternalOutput")

    with TileContext(nc) as tc:
        _body(nc, tc, T, debug, dbg)
    nc.compile()
    return nc


def _body(nc, tc, T, debug, dbg):
    extTn_d, refTt_d, refoT_d = T["extTn"], T["refTt"], T["refoT"]
    refnat_d, shT_d, extrows_d = T["refnat"], T["shT"], T["extrows"]
    ss12_d, mhalf_d, offtab_d, rrep_d = T["ss12"], T["mhalf"], T["offtab"], T["rrep"]
    partials = T["partials"]

    with tc.tile_pool(name="con", bufs=1) as con, \
         tc.tile_pool(name="shp", bufs=2) as shp, \
         tc.tile_pool(name="ps", bufs=1, space="PSUM") as psp, \
         tc.tile_pool(name="dram", bufs=1, space="DRAM") as dram:

        nc.gpsimd.load_library(library_config.mlp)

        # ---------- phase-1 inputs first: per-dc loads spread across queues ----------
        p1 = tc.alloc_tile_pool(name="p1", bufs=1)
        extTn_sb = [[p1.tile([128, ESH], BF16, name=f"extTn{b}{dc}", tag=f"extTn{b}{dc}")
                     for dc in range(DC)] for b in range(B)]
        refTt_sb = [[p1.tile([128, R], BF16, name=f"refTt{b}{dc}", tag=f"refTt{b}{dc}")
                     for dc in range(DC)] for b in range(B)]
        for b in range(B):
            for dc in range(DC):
                nc.sync.dma_start(out=extTn_sb[b][dc][:],
                                  in_=extTn_d[b, :, dc * ESH:(dc + 1) * ESH])
                nc.sync.dma_start(out=refTt_sb[b][dc][:],
                                  in_=refTt_d[b, :, dc * R:(dc + 1) * R])

        # ---------- constants / persistent ----------
        mh_sb = con.tile([2, 128], BF16, name="mh", tag="mh")
        nc.sync.dma_start(out=mh_sb[:], in_=mhalf_d[:])
        oft_sb = con.tile([128, 32], I32, name="oft", tag="oft")
        nc.sync.dma_start(out=oft_sb[:], in_=offtab_d[:])
        rrep_sb = con.tile([128, 4], F32, name="rrep", tag="rrep")
        nc.sync.dma_start(out=rrep_sb[:], in_=rrep_d[:])
        ss12_sb = con.tile([2, 12 * 512], BF16, name="ss12", tag="ss12")
        nc.sync.dma_start(out=ss12_sb[:], in_=ss12_d[:])
        refoT_sb = {}
        refnat_sb = {}
        for xi in range(2):
            for b in range(B):
                t1 = con.tile([128, DC * 128], BF16, name=f"refoT{xi}{b}", tag=f"refoT{xi}{b}")
                nc.sync.dma_start(out=t1[:], in_=refoT_d[xi, b])
                refoT_sb[(xi, b)] = t1
                t2 = con.tile([128, D], BF16, name=f"refnat{xi}{b}", tag=f"refnat{xi}{b}")
                nc.sync.dma_start(out=t2[:], in_=refnat_d[xi, b])
                refnat_sb[(xi, b)] = t2

        # shT stream pool: f=0 loads issued now, later f inside the loop.
        def load_shT(f):
            tiles = {}
            for xi in range(2):
                for b in range(B):
                    tl = shp.tile([128, DC * S], BF16, name=f"sh{xi}{b}", tag=f"sh{xi}{b}")
                    half = DC * S // 2
                    nc.sync.dma_start(out=tl[:, 0:half], in_=shT_d[f, xi, b, :, 0:half])
                    nc.sync.dma_start(out=tl[:, half:], in_=shT_d[f, xi, b, :, half:])
                    tiles[(xi, b)] = tl
            return tiles

        shT_sb = [None] * NFP
        shT_sb[0] = load_shT(0)

        scal = con.tile([128, 64], F32, name="scal", tag="scal")
        acc = con.tile([128, 20], F32, name="acc", tag="acc")
        it_sb = [con.tile([128, 16], I16, name=f"it{b}", tag=f"it{b}") for b in range(B)]
        go = [con.tile([128, 2, D], BF16, name=f"go{b}", tag=f"go{b}") for b in range(B)]
        hT = [con.tile([128, DC, RK], BF16, name=f"hT{b}", tag=f"hT{b}") for b in range(B)]

        a2a_in = [dram.tile([NC_N, 512], F32, name=f"a2a_in{b}", tag=f"a2a_in{b}")
                  for b in range(B)]
        a2a_out = [dram.tile([NC_N, 512], F32, name=f"a2a_out{b}", tag=f"a2a_out{b}")
                   for b in range(B)]
        widx = dram.tile([B, RSH, K], I16, name="widx", tag="widx")

        PP = 6  # psum tags, round-robined

        def psum_tile(idx):
            return psp.tile([128, 512], F32, name=f"pp{idx}", tag=f"pp{idx}")

        # ================= phase 1: sim + topk + pack =================
        if True:
            sim_sb = [p1.tile([128, ESH], F32, name=f"sim{b}", tag=f"sim{b}")
                      for b in range(B)]
            candp = [p1.tile([128, 16], F32, name=f"cand{rc}", tag=f"cand{rc}")
                     for rc in range(4)]
            g = 0
            for b in range(B):
                for rc in range(4):
                    for eh in range(2):
                        ps = psum_tile(g % PP)
                        g += 1
                        for dc in range(DC):
                            nc.tensor.matmul(
                                ps[:],
                                refTt_sb[b][dc][:, rc * 128:(rc + 1) * 128],
                                extTn_sb[b][dc][:, eh * 512:(eh + 1) * 512],
                                start=(dc == 0), stop=(dc == DC - 1))
                        nc.scalar.activation(sim_sb[b][:, eh * 512:(eh + 1) * 512],
                                             ps[:], AF.Copy)
                    mxt = p1.tile([128, 8], F32, name="mx", tag=f"mx{b}{rc}")
                    mit = p1.tile([128, 8], U32, name="mi", tag=f"mi{b}{rc}")
                    nc.vector.max(out=mxt[:], in_=sim_sb[b][:])
                    nc.vector.max_index(out=mit[:], in_max=mxt[:], in_values=sim_sb[b][:])
                    nc.vector.tensor_copy(candp[rc][:, b * 8:b * 8 + 4], mxt[:, 0:4])
                    nc.vector.tensor_copy(candp[rc][:, b * 8 + 4:b * 8 + 8].bitcast(U32),
                                          mit[:, 0:4])
                if debug:
                    nc.sync.dma_start(out=dbg["d_sim"][b], in_=sim_sb[b][:])
                # pack + exchange this b immediately: block j = [64 r, 8 c]
                for j in range(NC_N):
                    rc, half = j // 2, (j % 2) * 64
                    nc.sync.dma_start(
                        out=a2a_in[b][j, :].rearrange("(r c) -> r c", c=8),
                        in_=candp[rc][half:half + 64, b * 8:(b + 1) * 8])
                nc.gpsimd.collective_compute(
                    "AllToAll", OP.bypass, replica_groups=[list(range(NC_N))],
                    ins=[a2a_in[b][:]], outs=[a2a_out[b][:]])
        p1.release()

        # ================= angle phase pools =================
        with tc.tile_pool(name="p5", bufs=2) as p5, \
             tc.tile_pool(name="p5a", bufs=2) as p5a:

            # ---- sr matmuls for f=0 (overlap the AllToAll) ----
            sr_drained = {}

            def emit_sr(f):
                for xi in range(2):
                    for b in range(B):
                        blk = (f * 2 + xi) * 2 + b
                        ss_row = ss12_sb[0:2, blk * 512:(blk + 1) * 512]
                        ps = psum_tile(4 + b)
                        for dc in range(DC):
                            nc.tensor.matmul(ps[:],
                                             refoT_sb[(xi, b)][:, dc * 128:(dc + 1) * 128],
                                             shT_sb[f][(xi, b)][:, dc * S:(dc + 1) * S],
                                             start=(dc == 0), stop=False)
                        nc.tensor.matmul(ps[:], mh_sb[:], ss_row, start=False, stop=True)
                        # drains: sr2 = psum; isr = rsqrt(-2 psum + rr)
                        sr2, isr, _ = sr_drained[(f, xi)]
                        sl = slice(b * 512, (b + 1) * 512)
                        nc.scalar.activation(sr2[:, sl], ps[:], AF.Copy)
                        raw_activation(nc, isr[:, sl], ps[:], AF.Rsqrt,
                                       bias=rrep_sb[:, xi * 2 + b:xi * 2 + b + 1],
                                       scale=-2.0)
                        if RSQRT_NR:
                            ns2 = sr_drained[(f, xi)][2]
                            nc.scalar.activation(
                                ns2[:, sl], ps[:], AF.Identity, scale=-2.0,
                                bias=rrep_sb[:, xi * 2 + b:xi * 2 + b + 1])

            def alloc_sr(f):
                for xi in range(2):
                    sr2 = p5a.tile([128, 1024], BF16, name="sr2", tag=f"sr2_{xi}")
                    isr = p5a.tile([128, 1024], BF16, name="isr", tag=f"isr_{xi}")
                    ns2 = None
                    if RSQRT_NR:
                        ns2 = p5a.tile([128, 1024], BF16, name="ns2", tag=f"ns2_{xi}")
                    sr_drained[(f, xi)] = (sr2, isr, ns2)

            def nr_refine(y, x, tagp):
                # y <- y * (1.5 - 0.5 * x * y^2)
                t1 = p5a.tile([128, 1024], BF16, name="nrt", tag=f"nrt{tagp}")
                nc.vector.tensor_mul(t1[:], x[:], y[:])
                nc.vector.tensor_mul(t1[:], t1[:], y[:])
                nc.vector.tensor_scalar(t1[:], t1[:], -0.5, 1.5, OP.mult, OP.add)
                nc.vector.tensor_mul(y[:], y[:], t1[:])

            alloc_sr(0)
            emit_sr(0)
            shT_sb[1] = load_shT(1)
            alloc_sr(1)
            emit_sr(1)

            # ---- merge + gather + scal, per b (b=0 overlaps b=1's A2A) ----
            # scal columns: 0-3 hh(b,c) | 4-7 hh/2 | 8-15 rh(xi,b,c) | 16-23 ih |
            # 24-31 nih | 32-39 cih2 | 40-47 dih2 | 48-55 d' | 56-63 -rh
            for b in range(B):
                vi_b = con.tile([64, 64], F32, name=f"vi{b}", tag=f"vi{b}")
                for j in range(NC_N):
                    nc.sync.dma_start(
                        out=vi_b[:, j * 8:(j + 1) * 8],
                        in_=a2a_out[b][j, :].rearrange("(r c) -> r c", c=8))
                if debug:
                    nc.sync.dma_start(out=dbg["d_vi"][b * 64:(b + 1) * 64, :], in_=vi_b[:])
                vi3 = vi_b[:].rearrange("p (j c) -> p j c", c=8)
                v32 = con.tile([64, 32], F32, name=f"v32{b}", tag=f"v32{b}")
                nc.vector.tensor_copy(v32[:].rearrange("p (j k) -> p j k", k=4),
                                      vi3[:, :, 0:4])
                gidx = con.tile([64, 32], I32, name=f"gidx{b}", tag=f"gidx{b}")
                nc.vector.tensor_tensor(
                    gidx[:].rearrange("p (j k) -> p j k", k=4),
                    vi3[:, :, 4:8].bitcast(I32),
                    oft_sb[0:64, :].rearrange("p (j k) -> p j k", k=4), OP.add)
                gidxf = con.tile([64, 32], F32, name=f"gidxf{b}", tag=f"gidxf{b}")
                nc.vector.tensor_copy(gidxf[:], gidx[:])
                mv = con.tile([64, 8], F32, name=f"mv{b}", tag=f"mv{b}")
                nc.vector.max(out=mv[:], in_=v32[:])
                winf_b = con.tile([64, 4], F32, name=f"winf{b}", tag=f"winf{b}")
                for k in range(K):
                    msk = p5.tile([64, 32], F32, name="msk", tag="msk")
                    nc.vector.tensor_scalar(msk[:], v32[:], mv[:, k:k + 1], None,
                                            OP.is_equal)
                    junkC = p5.tile([64, 32], F32, name="junkC", tag="junkC")
                    nc.vector.scalar_tensor_tensor(junkC[:], gidxf[:], 0.0, msk[:],
                                                   OP.add, OP.mult,
                                                   accum_out=winf_b[:, k:k + 1])
                if debug:
                    nc.sync.dma_start(out=dbg["d_win"][b * 64:(b + 1) * 64, :],
                                      in_=winf_b[:])
                win16_b = con.tile([64, 4], I16, name=f"win16{b}", tag=f"win16{b}")
                nc.vector.tensor_copy(win16_b[:], winf_b[:])
                nc.sync.dma_start(out=widx[b], in_=win16_b[:])
                it16 = p5.tile([16, 16], I16, name="it16", tag=f"it16_{b}")
                for k in range(K):
                    nc.sync.dma_start(
                        out=it16[:, k * 4:(k + 1) * 4],
                        in_=widx[b].rearrange("(rh p) k -> p k rh", p=16)[:, k, :])
                for rep in range(8):
                    nc.sync.dma_start(out=it_sb[b][rep * 16:(rep + 1) * 16, :],
                                      in_=it16[:])
                nc.gpsimd.dma_gather(go[b][:], extrows_d[b], it_sb[b][:], RK, RK, D,
                                     single_packet=True)
                nc.gpsimd.dma_gather(hT[b][:], extrows_d[b], it_sb[b][:], RK, RK, D,
                                     transpose=True, single_packet=False)
                if debug:
                    cg = p5.tile([128, 2 * D], F32, name="dbgcp", tag="dbgcp", bufs=1)
                    nc.vector.tensor_copy(cg[:], go[b][:].rearrange("p c d -> p (c d)"))
                    nc.sync.dma_start(out=dbg["d_go"][b], in_=cg[:])
                    ch = p5.tile([128, DC * RK], F32, name="dbgcp", tag="dbgcp", bufs=1)
                    nc.vector.tensor_copy(ch[:], hT[b][:].rearrange("p q j -> p (q j)"))
                    nc.sync.dma_start(out=dbg["d_hT"][b], in_=ch[:])

                # per-b scal block
                for c in range(2):
                    junkB = p5.tile([128, D], BF16, name="junkB", tag="junkB")
                    nc.scalar.activation(junkB[:], go[b][:, c, :], AF.Square,
                                         accum_out=scal[:, b * 2 + c:b * 2 + c + 1])
                for xi in range(2):
                    for c in range(2):
                        col = 8 + xi * 4 + b * 2 + c
                        junkB = p5.tile([128, D], BF16, name="junkB", tag="junkB")
                        nc.vector.scalar_tensor_tensor(
                            junkB[:], go[b][:, c, :], 1.0, refnat_sb[(xi, b)][:],
                            OP.bypass, OP.mult, accum_out=scal[:, col:col + 1])
                hh2 = scal[:, b * 2:b * 2 + 2]
                hhh = scal[:, 4 + b * 2:6 + b * 2]
                nc.vector.tensor_scalar_mul(hhh, hh2, 0.5)
                for xi in range(2):
                    o = xi * 4 + b * 2
                    rh2 = scal[:, 8 + o:10 + o]
                    ih2 = scal[:, 16 + o:18 + o]
                    nih2 = scal[:, 24 + o:26 + o]
                    cih2 = scal[:, 32 + o:34 + o]
                    dih2 = scal[:, 40 + o:42 + o]
                    dp2 = scal[:, 48 + o:50 + o]
                    rrbc = rrep_sb[:, xi * 2 + b:xi * 2 + b + 1].to_broadcast([128, 2])
                    t1 = p5.tile([128, 2], F32, name="t1", tag="t1")
                    nc.vector.tensor_scalar_mul(t1[:], rh2, -2.0)
                    nc.vector.tensor_add(t1[:], t1[:], hh2)
                    t2 = p5.tile([128, 2], F32, name="t2", tag="t2")
                    nc.vector.tensor_tensor(t2[:], t1[:], rrbc, OP.add)  # nhr^2
                    nhr = p5.tile([128, 2], F32, name="nhr", tag="nhr")
                    nc.scalar.activation(nhr[:], t2[:], AF.Sqrt)
                    nc.vector.reciprocal(ih2, nhr[:])
                    nc.vector.tensor_scalar_mul(nih2, ih2, -1.0)
                    t3 = p5.tile([128, 2], F32, name="t3", tag="t3")
                    nc.vector.tensor_tensor(t3[:], rh2, rrbc, OP.subtract)  # rh-rr
                    nc.vector.tensor_sub(t3[:], hhh, t3[:])  # hh/2-rh+rr
                    nc.vector.tensor_mul(cih2, t3[:], ih2)
                    nc.vector.tensor_sub(dp2, hhh, rh2)      # d' = hh/2-rh
                    nc.vector.tensor_mul(dih2, dp2, ih2)
                    # cih2 += nih*hh/2 ; dih2 += ih*hh/2 ; nrh = -rh
                    t4 = p5.tile([128, 2], F32, name="t4", tag="t4")
                    nc.vector.tensor_mul(t4[:], nih2, hhh)
                    nc.vector.tensor_add(cih2, cih2, t4[:])
                    nc.vector.tensor_mul(t4[:], ih2, hhh)
                    nc.vector.tensor_add(dih2, dih2, t4[:])
                    nc.vector.tensor_scalar_mul(scal[:, 56 + o:58 + o], rh2, -1.0)
            if debug:
                nc.sync.dma_start(out=dbg["d_scal"][:], in_=scal[:])

            # ---------- angle grids ----------
            a_t = None
            for f in range(NFP):
                if f > 1:
                    alloc_sr(f)
                    emit_sr(f)
                if f == 1:
                    shT_sb[2] = load_shT(2)
                for xi in range(2):
                    sr2, isr, ns2 = sr_drained[(f, xi)]
                    u1 = [p5a.tile([128, 1024], BF16, name="u1", tag=f"u1_{c}") for c in range(2)]
                    ish = [p5a.tile([128, 1024], BF16, name="ish", tag=f"ish_{c}") for c in range(2)]
                    for b in range(B):
                        blk = (f * 2 + xi) * 2 + b
                        ss_row = ss12_sb[0:2, blk * 512:(blk + 1) * 512]
                        sl = slice(b * 512, (b + 1) * 512)
                        for c in range(2):
                            ps = psum_tile(b * 2 + c)
                            for dc in range(DC):
                                nc.tensor.matmul(ps[:],
                                                 hT[b][:, dc, c * 128:(c + 1) * 128],
                                                 shT_sb[f][(xi, b)][:, dc * S:(dc + 1) * S],
                                                 start=(dc == 0), stop=False)
                            nc.tensor.matmul(ps[:], mh_sb[:], ss_row, start=False, stop=True)
                            hhc = scal[:, b * 2 + c:b * 2 + c + 1]
                            nc.scalar.activation(u1[c][:, sl], ps[:], AF.Copy)
                            raw_activation(nc, ish[c][:, sl], ps[:], AF.Rsqrt,
                                           scale=-2.0, bias=hhc)
                    if debug and f == 0 and xi == 0:
                        for c in range(2):
                            uf = p5.tile([128, 1024], F32, name="dbgcp", tag="dbgcp", bufs=1)
                            nc.vector.tensor_copy(uf[:], u1[c][:])
                            nc.sync.dma_start(out=dbg["d_u1"][c], in_=uf[:])
                        sf_ = p5.tile([128, 1024], F32, name="dbgcp", tag="dbgcp", bufs=1)
                        nc.vector.tensor_copy(sf_[:], sr2[:])
                        nc.sync.dma_start(out=dbg["d_sr2"][0], in_=sf_[:])
                        sf2 = p5.tile([128, 1024], F32, name="dbgcp", tag="dbgcp", bufs=1)
                        nc.vector.tensor_copy(sf2[:], isr[:])
                        nc.sync.dma_start(out=dbg["d_sr2"][1], in_=sf2[:])

                    aj = []
                    for c in range(2):
                        t0 = p5a.tile([128, 1024], BF16, name="t0", tag=f"t0_{c}")
                        nc.vector.tensor_sub(t0[:], sr2[:], u1[c][:])
                        t1 = p5a.tile([128, 1024], BF16, name="t1g", tag=f"t1g_{c}")
                        nc.vector.tensor_add(t1[:], sr2[:], u1[c][:])
                        pp = p5a.tile([128, 1024], BF16, name="pp", tag="ppg")
                        nc.vector.tensor_mul(pp[:], isr[:], ish[c][:])
                        q = p5a.tile([128, 1024], BF16, name="q", tag="q")
                        pt = p5a.tile([128, 1024], BF16, name="pt", tag="pt")
                        w1 = p5a.tile([128, 1024], BF16, name="w1", tag="w1")
                        for b in range(B):
                            sl = slice(b * 512, (b + 1) * 512)
                            col = b * 2 + c
                            nc.vector.tensor_scalar(
                                q[:, sl], t0[:, sl],
                                scal[:, 24 + xi * 4 + col:25 + xi * 4 + col],
                                scal[:, 32 + xi * 4 + col:33 + xi * 4 + col],
                                OP.mult, OP.add)
                            nc.vector.tensor_scalar(
                                pt[:, sl], t0[:, sl],
                                scal[:, 16 + xi * 4 + col:17 + xi * 4 + col],
                                scal[:, 40 + xi * 4 + col:41 + xi * 4 + col],
                                OP.mult, OP.add)
                            nc.vector.tensor_scalar(
                                w1[:, sl], t1[:, sl],
                                scal[:, 56 + xi * 4 + col:57 + xi * 4 + col],
                                None, OP.add)
                        a1 = p5a.tile([128, 1024], BF16, name="a1", tag=f"a1_{xi}_{c}", bufs=1)
                        nc.vector.tensor_mul(a1[:], q[:], isr[:])
                        a2 = p5a.tile([128, 1024], BF16, name="a2", tag=f"a2_{xi}_{c}", bufs=1)
                        nc.vector.tensor_mul(a2[:], pt[:], ish[c][:])
                        a3 = p5a.tile([128, 1024], BF16, name="a3", tag=f"a3_{xi}_{c}", bufs=1)
                        nc.vector.tensor_mul(a3[:], w1[:], pp[:])
                        aj.append((a1, a2, a3))
                    if xi == 0:
                        a_t = aj
                        if debug and f == 0:
                            for c in range(2):
                                for jj in range(3):
                                    af_ = p5.tile([128, 1024], F32, name="dbgcp", tag="dbgcp", bufs=1)
                                    nc.vector.tensor_copy(af_[:], aj[c][jj][:])
                                    nc.sync.dma_start(out=dbg["d_a"][jj, c], in_=af_[:])
                    else:
                        for c in range(2):
                            for jj in range(3):
                                dj = p5.tile([128, 1024], BF16, name="dj", tag="dj")
                                nc.vector.tensor_sub(dj[:], aj[c][jj][:], a_t[c][jj][:])
                                slot = jj * 6 + f * 2 + c
                                if jj == 0:
                                    nc.vector.tensor_reduce(
                                        acc[:, slot:slot + 1], dj[:], AX.X, OP.add,
                                        apply_absolute_value=True)
                                else:
                                    junkB = p5.tile([128, D], BF16, name="junkB", tag="junkB")
                                    nc.scalar.activation(junkB[:], dj[:], AF.Abs,
                                                         accum_out=acc[:, slot:slot + 1])

            # ---------- final ----------
            accr = con.tile([128, 4], F32, name="accr", tag="accr")
            nc.vector.tensor_reduce(accr[:, 0:3],
                                    acc[:, 0:18].rearrange("p (j s) -> p j s", j=3),
                                    AX.X, OP.add)
            nc.vector.memset(accr[:, 3:4], 0.0)
            if debug:
                nc.sync.dma_start(out=dbg["d_acc"][:], in_=acc[:])
            par = con.tile([128, 4], F32, name="par", tag="par")
            nc.gpsimd.partition_all_reduce(par[:], accr[:], 128,
                                           bass_isa.ReduceOp.add)
            nc.sync.dma_start(out=partials[:], in_=par[0:1, :])


# ---------------- host side ----------------

def bf16(x):
    return np.asarray(x, dtype=ml_dtypes.bfloat16)


def prep_inputs(teacher_feats, student_feats, ref_perm, shared_perm):
    EXTRA_FRAMES = [1, 3, 5, 7]
    tf, sf = np.asarray(teacher_feats), np.asarray(student_feats)
    rp, sp = np.asarray(ref_perm), np.asarray(shared_perm)

    ref = np.stack([tf[:, 0, rp, :], sf[:, 0, rp, :]])          # [2,B,R,D] f32
    ext = np.concatenate([tf[:, f] for f in EXTRA_FRAMES], 1)   # [B,E,D] f32
    sh = np.stack([np.stack([tf[:, t, sp, :], sf[:, s, sp, :]])
                   for s, t in [(1, 2), (2, 4), (3, 6)]])       # [3,2,B,S,D] f32

    extn = ext / np.maximum(np.linalg.norm(ext, axis=-1, keepdims=True), 1e-12)
    # dc-packed transposes: [.., D, N] -> [.., DC, 128, N] -> [.., 128, DC*N]
    def dpack(x):  # x [..., N, D] -> [..., 128, DC*N]
        xt = np.swapaxes(x, -1, -2)                             # [..., D, N]
        shp = xt.shape[:-2]
        n = xt.shape[-1]
        xt = xt.reshape(*shp, DC, 128, n)
        xt = np.swapaxes(xt, -3, -2)                            # [..., 128, DC, n]
        return np.ascontiguousarray(xt.reshape(*shp, 128, DC * n))

    extn_p = dpack(bf16(extn))                                  # [B,128,DC*E]
    refTt_p = dpack(bf16(ref[0]))                               # [B,128,DC*R]
    shT_p = dpack(bf16(sh))                                     # [3,2,B,128,DC*S]

    ss = np.sum(sh.astype(np.float64) * sh, axis=-1)            # [3,2,B,S]
    ss12 = np.zeros((2, 12 * 512), dtype=ml_dtypes.bfloat16)
    ss12[0] = bf16(ss.reshape(-1))
    rrf = np.sum(ref.astype(np.float64) * ref, axis=-1)         # [2,B,R]

    mhalf = np.zeros((2, 128), dtype=ml_dtypes.bfloat16)
    mhalf[0] = -0.5
    offtab = np.broadcast_to((np.arange(32) // 4 * ESH).astype(np.int32),
                             (128, 32)).copy()

    extb = bf16(ext)
    in_maps = []
    for c in range(NC_N):
        rs = slice(c * RSH, (c + 1) * RSH)
        esl = slice(c * ESH, (c + 1) * ESH)
        # extTn shard: cols dc*ESH+e from full dc*E+
        extn_sh = extn_p.reshape(B, 128, DC, E)[:, :, :, esl].reshape(B, 128, DC * ESH)
        refo = ref[:, :, rs, :]                                  # [2,B,64,D]
        reps = np.concatenate([refo, refo], axis=2)              # [2,B,128,D]
        refoT = dpack(bf16(reps))                                # [2,B,128,DC*128]
        rrep = np.ascontiguousarray(
            np.concatenate([rrf[:, :, rs], rrf[:, :, rs]], axis=2)  # [2,B,128]
            .reshape(4, 128).T.astype(np.float32))               # [128,4] col=xi*2+b
        m = {
            "extTn": np.ascontiguousarray(extn_sh),
            "refTt": refTt_p,
            "refoT": refoT,
            "refnat": bf16(reps),
            "shT": shT_p,
            "extrows": extb,
            "ss12": ss12, "mhalf": mhalf, "offtab": offtab, "rrep": rrep,
        }
        in_maps.append(m)
    return in_maps


_NC_CACHE = {}


def kernel(teacher_feats, student_feats, ref_perm, shared_perm,
           debug=False, trace=False, use_sim=False):
    key = ("nc", debug)
    if key not in _NC_CACHE:
        _NC_CACHE[key] = build(debug=debug)
    nc = _NC_CACHE[key]
    in_maps = prep_inputs(teacher_feats, student_feats, ref_perm, shared_perm)
    if use_sim:
        from concourse.bass_interp import MultiCoreSim
        nc.insert_bir_kernel_barrier_sem_inc()
        sim = MultiCoreSim(nc, NC_N)
        for t in range(NC_N):
            for name, arr in in_maps[t].items():
                sim.cores[t].tensor(name)[:] = arr
        sim.simulate()
        out_names = ["partials"] + (
            [k for k in ("d_sim", "d_vi", "d_win", "d_go", "d_hT", "d_scal",
                         "d_u1", "d_sr2", "d_a", "d_acc")] if debug else [])
        results = [{name: np.array(sim.cores[t].tensor(name)) for name in out_names}
                   for t in range(NC_N)]

        class _R:
            pass
        res = _R()
        res.results = results
        res.exec_time_ns = None
    else:
        res = run_bass_kernel_spmd(nc, in_maps, list(range(NC_N)), trace=trace)
    parts = np.stack([res.results[c]["partials"][0, :3] for c in range(NC_N)])
    total = B * R * S * K * 3
    loss = np.float32(parts.sum() / total)
    if debug or trace:
        return loss, res
    return loss
